# revision 56
# baseline (speedup 1.0000x reference)
"""Trainium2 Bass kernel: exact 3D Euclidean distance transform of a binary
(16, 512, 512) float32 volume — distance from every nonzero voxel to the
nearest zero voxel over ALL three axes (batch participates in the metric),
matching scipy.ndimage.distance_transform_edt on the full array.

Fast path / slow path split:
  Device (this kernel): separable EDT with an exact W pass (fwd/bwd
  saturating scans) and parabola min-plus passes along H and B banded at
  radius R=2.  This is exact for every voxel whose true distance is < R+1
  (its optimal per-axis offsets are <= floor(d) <= R), i.e. for ~99% of
  voxels at the 5%-background density this module targets.
  Host: every voxel with device d^2 >= (R+1)^2 (any voxel the band could
  have gotten wrong necessarily lands in this set, because the banded value
  only ever over-estimates and a band violation implies true d >= R+1) is
  re-solved exactly by a vectorized radius-6 window search; if any such
  voxel has no zero within distance < 6 the whole volume falls back to an
  exact host EDT.  The patched result is exact everywhere, for any input.

Device pipeline (values are small integers <= CLAMP^2+8, exact in fp16,
which unlocks the DVE 2x/4x perf modes):
  pass W: 1D nearest-zero distance along W via fwd/bwd scans
          (tensor_tensor_scan, DVE-only op), squared during the PSUM
          evacuation of a PE transpose (ACT Square).
  pass H: banded parabola min-plus along H (radius 2).
  pass B: banded parabola min-plus along B (radius 2).
  Output is d^2 in fp16, w-major; the host does the final sqrt.

Engine split (only DVE and ACT can do general elementwise work on TRN2
silicon; Pool rejects TensorTensor/TensorScalarPtr at codegen):
  DVE: scans (1x), every min (tensor_tensor, 2x), the B-pass +1 adds
       (tensor_scalar, 4x).
  ACT: PSUM evacuation fused with Square, +s^2 adds (Copy + bias).
  PE:  transposes.  SP(sync): HWDGE DMA issue.  Pool: constants and the
       SWDGE strip DMAs.  Binarize happens on the host ((x != 0) * CLAMP —
       the f16 wire format — which also makes NaN inputs foreground, as in
       the reference).
The H pass is split into (b-half x j-half) quarters whose b-halves align
with the two PSUM-evacuation groups; pairs are emitted before folds so the
serial fold chains never starve; the B pass runs per j-half / per j so
each output DMA starts as soon as its slice is final.

Sharding: data-parallel over H (8 slabs of 64 rows, NO halo); the W-scan
needs full W and the B-pass full B, which each slab has.  The H pass only
produces rows R..HS-R of each slab; the 4 rows around every slab boundary
(and the 2 at each volume edge) are re-solved exactly on the host from the
exported squared-W-distance strips (st5), before the far-voxel patch.  No
cross-core communication.

Hardware quirk: several instruction encodings accept only ONE semaphore
wait; _split_multi_waits hoists extra waits onto same-engine NoOp carriers.
"""
import numpy as np

B, H, W = 16, 512, 512
NCORES = 8
HS = H // NCORES          # 64 interior rows per core
P = 128
CLAMP = 32.0
R = 2                     # band radius of the H and B passes

HB = 0                    # no input halo: the host re-solves the 4 rows
                          # around each slab boundary (and the volume edges)
HE = HS + 2 * HB          # 64 rows per core
N_T = (B * HE) // P       # 8 scan tiles
N_J = W // P              # 4 w-groups
CE = B * HE               # 1024 transposed lines per j-group
C = B * HS                # 1024 interior (b,h) elements per j-group
NG = 4                    # PSUM evacuation groups per j (2 tiles each)

_BUILT = None
LAST_RESULTS = []   # kept for the test harness's profiling hook


def _k5_body(tc, out_d, st_d, xs_d):
    """Fused single-launch banded-EDT device pass.

    xs_d:  [16, HE, 512] f16 dram (ExternalInput, host-binarized h-slab)
    out_d: [512, 16, HS] f16 dram (ExternalOutput), squared distances,
           w-major.  Rows h in {0,1,62,63} of each slab carry garbage (no
    halo); the host re-solves them from st_d.
    st_d:  [512, 16, 8] f16 dram (ExternalOutput): the squared W-distances
           of rows h in {0..3, 60..63} (what the host boundary fix needs).
    """
    import concourse.mybir as mybir

    nc = tc.nc
    f16 = mybir.dt.float16
    Alu = mybir.AluOpType
    Act = mybir.ActivationFunctionType

    from concourse.masks import make_identity

    with tc.tile_pool(name="const", bufs=1) as cpool, \
         tc.tile_pool(name="big", bufs=1) as bpool, \
         tc.tile_pool(name="htmp", bufs=12) as hpool, \
         tc.tile_pool(name="btmp", bufs=6) as tbpool, \
         tc.tile_pool(name="psum", bufs=4, space="PSUM") as ppool, \
         tc.tile_pool(name="psumw", bufs=1, space="PSUM") as ppoolw:

        ones = cpool.tile([P, W], f16)
        nc.gpsimd.memset(ones[:], 1.0)
        ident = cpool.tile([P, P], f16)
        make_identity(nc, ident[:])
        # dummy transpose so PE observes the gpsimd-built identity before the
        # real transposes (keeps every matmul at <= 1 semaphore wait)
        psw = ppoolw.tile([P, P], f16)
        nc.tensor.transpose(psw[:], ident[:], ident[:])

        AALL = bpool.tile([P, N_T * W], f16)    # d0 = (x != 0) * CLAMP (host)
        FALL = bpool.tile([P, N_T * W], f16)    # fwd scan
        DALL = bpool.tile([P, N_T * W], f16)    # bwd scan of fwd = 1D dist

        # input DMAs on the sync queue (its trigger issue starts immediately;
        # the Pool queue is busy building the identity).  The host sends the
        # already-binarized (x != 0) * CLAMP field, so the first scan starts
        # as soon as the first (single-tile) chunk lands.
        xflat = xs_d.rearrange("b h w -> (b h) w")
        chunks = [(0, 1), (1, 2), (3, 2), (5, 3)]
        for t0, k in chunks:
            if k == 1:
                nc.sync.dma_start(AALL[:, W * t0: W * (t0 + 1)],
                                  xflat[P * t0: P * (t0 + 1)])
            else:
                nc.sync.dma_start(
                    AALL[:, W * t0: W * (t0 + k)].rearrange(
                        "p (g w) -> p g w", g=k),
                    xflat[P * t0: P * (t0 + k)].rearrange(
                        "(g pp) w -> pp g w", g=k))

        for t in range(N_T):
            fa = FALL[:, W * t: W * (t + 1)]
            nc.vector.tensor_tensor_scan(
                fa, ones[:, 0:W], AALL[:, W * t: W * (t + 1)], CLAMP,
                Alu.add, Alu.min)
            nc.vector.tensor_tensor_scan(
                DALL[:, W * t: W * (t + 1)][:, ::-1], ones[:, 0:W],
                fa[:, ::-1], CLAMP, Alu.add, Alu.min)

        # transpose + evacuate-with-Square, in NG groups of 4 scan tiles per
        # j-group; group g holds exactly the b-half g (4*128 = 8*64 lines).
        SQ = bpool.tile([P, N_J * CE], f16)     # w lines x (j, b, h64)
        GT = N_T // NG                          # 4 tiles per group
        for g in range(NG):
            for j in range(N_J):
                ps = ppool.tile([P, GT * P], f16, tag="ps")
                for tt in range(GT):
                    t = g * GT + tt
                    nc.tensor.transpose(
                        ps[:, P * tt: P * (tt + 1)],
                        DALL[:, W * t + P * j: W * t + P * (j + 1)],
                        ident[:])
                nc.scalar.activation(
                    SQ[:, CE * j + GT * P * g: CE * j + GT * P * (g + 1)],
                    ps[:], Act.Square)

        sq5 = SQ[:].rearrange("p (j b h) -> p j b h", j=N_J, b=B)
        ACH = bpool.tile([P, N_J * C], f16)
        ah4 = ACH[:].rearrange("p (j b h) -> p j b h", j=N_J, b=B)
        ACC = bpool.tile([P, N_J * C], f16)
        ac4 = ACC[:].rearrange("p (j b h) -> p j b h", j=N_J, b=B)
        # rows h in {0,1,62,63} are host-re-solved; give them a defined value
        # so the B pass math on those columns stays finite.
        nc.gpsimd.memset(ACH[:], 1024.0)

        # the squared-W-distance strips the host boundary fix needs: rows
        # h in {0..3} and {60..63}.  SWDGE (Pool queue) keeps these off the
        # 8 HWDGE lanes; they are host-bound, not device-critical.
        stv = st_d.rearrange("(j p) b e -> p j b e", p=P)
        for j in range(N_J):
            nc.gpsimd.dma_start(stv[:, j, :, 0:4], sq5[:, j, :, 0:4])
            nc.gpsimd.dma_start(stv[:, j, :, 4:8], sq5[:, j, :, HS - 4:HS])

        # b-halves align with the two PSUM evacuation groups.
        BCH = [(0, 8), (8, 16)]
        HV = HS - 2 * R           # 60 valid output rows, h in [R, HS-R)

        def h_pairs(bc, jh):
            """Pass H pair mins (DVE) + in-place +s^2 (ACT) on one
            (b-half, j-half).  All pairs are emitted before any fold so the
            DVE fold chains never starve."""
            b0, b1 = BCH[bc]
            nb = b1 - b0
            sq = sq5[:, 2 * jh:2 * (jh + 1), b0:b1, :]
            ts = []
            for s in range(1, R + 1):
                lo = sq[:, :, :, R - s:R - s + HV]
                hi = sq[:, :, :, R + s:R + s + HV]
                t_ = hpool.tile([P, 2 * nb * HV], f16, tag="hq")
                tv = t_[:].rearrange("p (j b h) -> p j b h", j=2, b=nb)
                nc.vector.tensor_tensor(tv, lo, hi, Alu.min)
                ts.append(tv)
            for s in range(1, R + 1):
                nc.scalar.activation(ts[s - 1], ts[s - 1], Act.Copy,
                                     bias=float(s * s))
            return ts

        def h_folds(bc, jh, ts):
            """Pass H fold chain (DVE) on one (b-half, j-half)."""
            b0, b1 = BCH[bc]
            sq = sq5[:, 2 * jh:2 * (jh + 1), b0:b1, :]
            a = ah4[:, 2 * jh:2 * (jh + 1), b0:b1, R:R + HV]
            ctr = sq[:, :, :, R:R + HV]
            nc.vector.tensor_tensor(a, ts[0], ctr, Alu.min)
            for s in range(2, R + 1):
                nc.vector.tensor_tensor(a, ts[s - 1], a, Alu.min)

        def b_adds(j0, nj):
            """ACT-side prep for pass B on j-groups [j0, j0+nj): the b = B-1
            strip of the accumulator (its center term) and the shared +s^2
            tensors for s >= 2 (s = 1 is a DVE 4x tensor_scalar in
            b_folds)."""
            a = ah4[:, j0:j0 + nj]
            c = ac4[:, j0:j0 + nj]
            nc.scalar.activation(c[:, :, B - 1:B, :], a[:, :, B - 1:B, :],
                                 Act.Copy, bias=0.0)
            tbs = []
            for s in range(2, R + 1):
                tb = tbpool.tile([P, nj * C], f16, tag=f"tb{nj}")
                tv = tb[:].rearrange("p (j b h) -> p j b h", j=nj, b=B)
                nc.scalar.activation(tv, a, Act.Copy, bias=float(s * s))
                tbs.append(tv)
            return tbs

        def b_folds(j0, nj, tbs, h0=0, h1=HS):
            """Pass B directional folds on j-groups [j0, j0+nj), h-range
            [h0, h1) (DVE).  The s=1 add runs on DVE (4x) so the chain
            starts without ACT."""
            a = ah4[:, j0:j0 + nj, :, h0:h1]
            c = ac4[:, j0:j0 + nj, :, h0:h1]
            tb1 = tbpool.tile([P, nj * B * (h1 - h0)], f16, tag=f"tbd{nj}")
            t1 = tb1[:].rearrange("p (j b h) -> p j b h", j=nj, b=B)
            nc.vector.tensor_scalar(t1, a, 1.0, None, Alu.add)
            for s in range(1, R + 1):
                tv = t1 if s == 1 else tbs[s - 2][:, :, :, h0:h1]
                bc = B - s
                if s == 1:
                    nc.vector.tensor_tensor(c[:, :, 0:bc, :],
                                            tv[:, :, s:B, :],
                                            a[:, :, 0:bc, :], Alu.min)
                else:
                    nc.vector.tensor_tensor(c[:, :, 0:bc, :],
                                            tv[:, :, s:B, :],
                                            c[:, :, 0:bc, :], Alu.min)
                nc.vector.tensor_tensor(c[:, :, s:B, :], tv[:, :, 0:bc, :],
                                        c[:, :, s:B, :], Alu.min)

        outd = out_d.rearrange("(j p) b h -> p j (b h)", p=P)
        accs = ACC[:].rearrange("p (j c) -> p j c", j=N_J)

        # DVE order keeps the engine stall-free: every chunk's pairs first
        # (they only depend on evacuations), then the jh0 fold chains, the
        # jh0 B pass + its DMA, the jh1 fold chains, then per-j B chains
        # each followed by its own DMA so the tail drains incrementally.
        ts = {}
        for bc, jh in [(0, 0), (1, 0), (0, 1), (1, 1)]:
            ts[(bc, jh)] = h_pairs(bc, jh)
        h_folds(0, 0, ts[(0, 0)])
        h_folds(1, 0, ts[(1, 0)])
        tbs0 = b_adds(0, 2)
        b_folds(0, 2, tbs0)
        nc.sync.dma_start(outd[:, 0:2], accs[:, 0:2])
        h_folds(0, 1, ts[(0, 1)])
        h_folds(1, 1, ts[(1, 1)])
        tbs23 = b_adds(2, 2)
        b_folds(2, 1, [t[:, 0:1] for t in tbs23])
        nc.sync.dma_start(outd[:, 2:3], accs[:, 2:3])
        b_folds(3, 1, [t[:, 1:2] for t in tbs23])
        nc.sync.dma_start(outd[:, 3:4], accs[:, 3:4])


def _split_multi_waits(nc):
    """Walrus in this toolchain encodes at most ONE sync wait per hardware
    instruction.  Hoist extra waits onto same-engine NoOp carriers inserted
    immediately before the over-subscribed instruction (program order on the
    engine preserves the semantics exactly)."""
    import concourse.mybir as mybir

    n = 0
    for fn in nc.m.functions:
        for blk in fn.blocks:
            insts = blk.instructions
            out = []
            for inst in insts:
                si = inst.sync_info
                if si is not None and len(si.on_wait) > 1:
                    waits = list(si.on_wait)
                    for w in waits[:-1]:
                        nop = mybir.InstNoOp(
                            name=f"waitsplit-{n}", ins=[], outs=[])
                        n += 1
                        nop.engine = inst.engine
                        nop.sync_info = mybir.SyncInfo(
                            on_wait=[w], on_update=[])
                        out.append(nop)
                    inst.sync_info = mybir.SyncInfo(
                        on_wait=[waits[-1]], on_update=list(si.on_update))
                out.append(inst)
            blk.instructions = out
    return n


def _make_tc_class():
    """TileContext whose kernel-tail drain is split into one drain per proc.

    The stock tail emits a single sync-engine Drain waiting on every
    outstanding processor; this walrus build only encodes ONE sync wait per
    instruction, so the aggregated drain fails codegen.  Semantics are
    identical — the waits just land on consecutive Drain instructions.
    """
    import concourse.tile as tile
    from concourse.vector_clock import ScopedClock, VectorClock

    class SplitDrainTileContext(tile.TileContext):
        def _drain_and_barrier(self, tick_clock, wait_clock):
            gvc = tick_clock.global_clock
            for proc in range(len(gvc)):
                t = gvc[proc]
                if t <= 0:
                    continue
                d = self.nc.sync.drain()
                sv = VectorClock([0] * len(gvc))
                sv.require_at_least(proc, t)
                wait_clock.add_sem_waits(d.ins, ScopedClock({None: sv}))
            self.nc.all_engine_barrier()
            assert self.sems is not None
            popped = self.nc._tile_sem_poison_stack.pop()
            assert popped is self._sem_poison
            self.nc.clear_and_free_semaphores(
                list(self.sems.allocated().values()))
            self.nc.all_engine_barrier()

    return SplitDrainTileContext


def _build():
    """Build the fused Bass module (done once per process)."""
    import concourse.bass as bass
    import concourse.mybir as mybir

    f16 = mybir.dt.float16
    TC = _make_tc_class()

    nc5 = bass.Bass("TRN2", debug=False, num_devices=NCORES)
    xs5_d = nc5.dram_tensor("xs5", [B, HE, W], f16,
                            kind="ExternalInput").ap()
    ot5_d = nc5.dram_tensor("ot5", [W, B, HS], f16,
                            kind="ExternalOutput").ap()
    st5_d = nc5.dram_tensor("st5", [W, B, 8], f16,
                            kind="ExternalOutput").ap()
    with TC(nc5) as tc:
        _k5_body(tc, ot5_d, st5_d, xs5_d)
    _split_multi_waits(nc5)
    return (nc5,)


def _host_exact_edt(x):
    """Exact host fallback: banded numpy EDT with growing radius (f32)."""
    INF = np.float32(1e9)
    r = 8
    while True:
        d0 = np.where(x != 0, INF, np.float32(0.0))
        fwd = np.empty_like(d0)
        st = np.full(d0.shape[:2], INF, np.float32)
        for w in range(W):
            st = np.minimum(st + 1.0, d0[:, :, w]); fwd[:, :, w] = st
        st = np.full(d0.shape[:2], INF, np.float32)
        bwd = np.empty_like(d0)
        for w in range(W - 1, -1, -1):
            st = np.minimum(st + 1.0, d0[:, :, w]); bwd[:, :, w] = st
        d2 = np.minimum(fwd, bwd) ** 2
        for axis in (0, 1):
            src = d2
            acc = src.copy()
            rr = min(r, x.shape[axis] - 1)
            for s in range(1, rr + 1):
                sl_lo = [slice(None)] * 3
                sl_hi = [slice(None)] * 3
                sl_lo[axis] = slice(0, x.shape[axis] - s)
                sl_hi[axis] = slice(s, None)
                np.minimum(acc[tuple(sl_lo)], src[tuple(sl_hi)] + s * s,
                           out=acc[tuple(sl_lo)])
                np.minimum(acc[tuple(sl_hi)], src[tuple(sl_lo)] + s * s,
                           out=acc[tuple(sl_hi)])
            d2 = acc
        out = np.sqrt(d2)
        # exact when every per-axis offset fits in the band; r >= max dim
        # means the bands are complete regardless of the value of out
        if out.max() <= r or r >= max(x.shape):
            return out.astype(np.float32)
        r *= 2


_RUNNER = None


def _make_runner(nc, n_cores):
    """Build the sharded PJRT callable once (run_bass_kernel_spmd re-traces
    and re-jits on every call; caching saves ~1 s per kernel() invocation)."""
    import jax
    import numpy as _np
    from jax.sharding import Mesh, PartitionSpec
    from jax.experimental.shard_map import shard_map
    import concourse.mybir as mybir
    from concourse import bass2jax

    bass2jax.install_neuronx_cc_hook()
    partition_name = (nc.partition_id_tensor.name
                      if nc.partition_id_tensor else None)
    in_names, out_names, out_avals, zero_outs = [], [], [], []
    for alloc in nc.m.functions[0].allocations:
        if not isinstance(alloc, mybir.MemoryLocationSet):
            continue
        name = alloc.memorylocations[0].name
        if alloc.kind == "ExternalInput":
            if name != partition_name:
                in_names.append(name)
        elif alloc.kind == "ExternalOutput":
            out_avals.append(jax.core.ShapedArray(
                tuple(alloc.tensor_shape), mybir.dt.np(alloc.dtype)))
            out_names.append(name)
            zero_outs.append(_np.zeros(tuple(alloc.tensor_shape),
                                       mybir.dt.np(alloc.dtype)))
    all_in = list(in_names) + list(out_names)
    if partition_name is not None:
        all_in.append(partition_name)

    def _body(*args):
        operands = list(args)
        if partition_name is not None:
            operands.append(bass2jax.partition_id_tensor())
        return tuple(bass2jax._bass_exec_p.bind(
            *operands, out_avals=tuple(out_avals), in_names=tuple(all_in),
            out_names=tuple(out_names), lowering_input_output_aliases=(),
            sim_require_finite=True, sim_require_nnan=True, nc=nc))

    devices = jax.devices()[:n_cores]
    mesh = Mesh(_np.asarray(devices), ("core",))
    n_io = len(in_names) + len(out_names)
    fn = jax.jit(shard_map(_body, mesh=mesh,
                           in_specs=(PartitionSpec("core"),) * n_io,
                           out_specs=(PartitionSpec("core"),) * len(out_names),
                           check_rep=False), keep_unused=True)

    def run(in_maps):
        concat_in = [_np.concatenate([_np.asarray(in_maps[c][n])
                                      for c in range(n_cores)], axis=0)
                     for n in in_names]
        concat_zero = [_np.zeros((n_cores * z.shape[0], *z.shape[1:]), z.dtype)
                       for z in zero_outs]
        outs = fn(*concat_in, *concat_zero)
        return [{name: _np.asarray(outs[i]).reshape(
                    n_cores, *out_avals[i].shape)[c]
                 for i, name in enumerate(out_names)}
                for c in range(n_cores)]

    return run


def _fix_boundaries(d2, results):
    """Re-solve the 4 rows around every slab boundary (and the 2 rows at
    each volume edge) from the exported squared-W-distance strips: the
    device ran the H pass without halo, so those rows are garbage.  This is
    the same radius-R H+B min-plus the device does, just in numpy on 32 of
    512 rows."""
    # dw2[b, h, w] for the strip rows: slab k locals {0..3} u {60..63}
    dw2 = np.full((B, H, W), np.float32(np.inf))
    for k in range(NCORES):
        st = np.asarray(results[k]["st5"]).astype(np.float32)  # [W, B, 8]
        st = st.transpose(1, 2, 0)                             # [B, 8, W]
        dw2[:, k * HS:k * HS + 4, :] = st[:, 0:4, :]
        dw2[:, k * HS + HS - 4:k * HS + HS, :] = st[:, 4:8, :]
    rows = sorted({r for m in range(NCORES + 1)
                   for r in (m * HS - 2, m * HS - 1, m * HS, m * HS + 1)
                   if 0 <= r < H})
    for hg in rows:
        acc = None
        for dh in range(-R, R + 1):
            hh = hg + dh
            if not 0 <= hh < H:
                continue
            cand = dw2[:, hh, :] + np.float32(dh * dh)
            acc = cand if acc is None else np.minimum(acc, cand)
        # pass B (radius R) along the batch axis
        accb = acc.copy()
        for db in range(1, R + 1):
            np.minimum(accb[:-db], acc[db:] + db * db, out=accb[:-db])
            np.minimum(accb[db:], acc[:-db] + db * db, out=accb[db:])
        d2[:, hg, :] = accb


def _patch_far(d2, xin):
    """Re-solve every voxel with banded d^2 >= (R+1)^2 exactly via a
    radius-6 window search (any voxel the band could have gotten wrong is in
    this set: the banded value only over-estimates, and a band violation
    implies true distance >= R+1).  Returns (patched d2, ok); ok=False means
    some such voxel has no zero within distance < 6 (or there are
    implausibly many) and the caller must use the full exact fallback."""
    sus = np.argwhere(d2 >= (R + 1) ** 2 - 0.5)
    if sus.shape[0] == 0:
        return d2, True
    if sus.shape[0] > 1_000_000:
        return d2, False
    rr = 6
    zp = np.pad(xin == 0, rr, constant_values=False)
    og = np.arange(-rr, rr + 1, dtype=np.int32)
    ob, oh, ow = np.meshgrid(og, og, og, indexing="ij")
    w2 = (ob * ob + oh * oh + ow * ow).astype(np.float32).ravel()
    obf = (ob.ravel() + rr)[None, :]
    ohf = (oh.ravel() + rr)[None, :]
    owf = (ow.ravel() + rr)[None, :]
    vals = np.empty(sus.shape[0], np.float32)
    CH = 2048
    for i0 in range(0, sus.shape[0], CH):
        s = sus[i0:i0 + CH].astype(np.int32)
        win = zp[s[:, 0:1] + obf, s[:, 1:2] + ohf, s[:, 2:3] + owf]
        d2w = np.where(win, w2[None, :], np.float32(1e9)).min(axis=1)
        if (d2w > 35.5).any():
            return d2, False
        vals[i0:i0 + CH] = d2w
    d2[sus[:, 0], sus[:, 1], sus[:, 2]] = vals
    return d2, True


def kernel(x):
    global _BUILT, _RUNNER
    x = np.asarray(x)
    assert x.shape == (B, H, W)
    if x.dtype != np.float32:
        x = x.astype(np.float32)

    if _BUILT is None:
        _BUILT = _build()
    (nc5,) = _BUILT
    if _RUNNER is None:
        _RUNNER = _make_runner(nc5, NCORES)
    LAST_RESULTS.clear()

    nan_mask = np.isnan(x)
    # The host binarizes: the device receives (x != 0) * CLAMP directly
    # (NaN != 0 is True, so NaN voxels are foreground, as in the reference).
    xin = (x != 0).astype(np.float32)     # 0 at zeros, 1 at foreground/NaN
    xp = (xin * CLAMP).astype(np.float16)
    in5 = [{"xs5": np.ascontiguousarray(xp[:, k * HS:k * HS + HE, :])}
           for k in range(NCORES)]
    results = _RUNNER(in5)
    outt = np.concatenate([results[k]["ot5"] for k in range(NCORES)], axis=2)

    d2 = outt.transpose(1, 2, 0).astype(np.float32)   # (w,b,h) -> (b,h,w)
    _fix_boundaries(d2, results)
    d2, ok = _patch_far(d2, xin)
    out = np.sqrt(d2) if ok else _host_exact_edt(xin)

    if nan_mask.any():
        out = np.where(nan_mask, np.float32(np.nan), out)
    return out


# revision 57
# speedup vs baseline: 1.0055x; 1.0055x over previous
"""Trainium2 Bass kernel: exact 3D Euclidean distance transform of a binary
(16, 512, 512) float32 volume — distance from every nonzero voxel to the
nearest zero voxel over ALL three axes (batch participates in the metric),
matching scipy.ndimage.distance_transform_edt on the full array.

Fast path / slow path split:
  Device (this kernel): separable EDT with an exact W pass (fwd/bwd
  saturating scans) and parabola min-plus passes along H and B banded at
  radius R=2.  This is exact for every voxel whose true distance is < R+1
  (its optimal per-axis offsets are <= floor(d) <= R), i.e. for ~99% of
  voxels at the 5%-background density this module targets.
  Host: every voxel with device d^2 >= (R+1)^2 (any voxel the band could
  have gotten wrong necessarily lands in this set, because the banded value
  only ever over-estimates and a band violation implies true d >= R+1) is
  re-solved exactly by a vectorized radius-6 window search; if any such
  voxel has no zero within distance < 6 the whole volume falls back to an
  exact host EDT.  The patched result is exact everywhere, for any input.

Device pipeline (values are small integers <= CLAMP^2+8, exact in fp16,
which unlocks the DVE 2x/4x perf modes):
  pass W: 1D nearest-zero distance along W via fwd/bwd scans
          (tensor_tensor_scan, DVE-only op), squared during the PSUM
          evacuation of a PE transpose (ACT Square).
  pass H: banded parabola min-plus along H (radius 2).
  pass B: banded parabola min-plus along B (radius 2).
  Output is d^2 in fp16, w-major; the host does the final sqrt.

Engine split (only DVE and ACT can do general elementwise work on TRN2
silicon; Pool rejects TensorTensor/TensorScalarPtr at codegen):
  DVE: scans (1x), every min (tensor_tensor, 2x), the B-pass +1 adds
       (tensor_scalar, 4x).
  ACT: PSUM evacuation fused with Square, +s^2 adds (Copy + bias).
  PE:  transposes.  SP(sync): HWDGE DMA issue.  Pool: constants and the
       SWDGE strip DMAs.  Binarize happens on the host ((x != 0) * CLAMP —
       the f16 wire format — which also makes NaN inputs foreground, as in
       the reference).
The H pass is split into (b-half x j-half) quarters whose b-halves align
with the two PSUM-evacuation groups; pairs are emitted before folds so the
serial fold chains never starve; the B pass runs per j-half / per j so
each output DMA starts as soon as its slice is final.

Sharding: data-parallel over H (8 slabs of 64 rows, NO halo); the W-scan
needs full W and the B-pass full B, which each slab has.  The H pass only
produces rows R..HS-R of each slab; the 4 rows around every slab boundary
(and the 2 at each volume edge) are re-solved exactly on the host from the
exported squared-W-distance strips (st5), before the far-voxel patch.  No
cross-core communication.

Hardware quirk: several instruction encodings accept only ONE semaphore
wait; _split_multi_waits hoists extra waits onto same-engine NoOp carriers.
"""
import numpy as np

B, H, W = 16, 512, 512
NCORES = 8
HS = H // NCORES          # 64 interior rows per core
P = 128
CLAMP = 32.0
R = 2                     # band radius of the H and B passes

HB = 0                    # no input halo: the host re-solves the 4 rows
                          # around each slab boundary (and the volume edges)
HE = HS + 2 * HB          # 64 rows per core
N_T = (B * HE) // P       # 8 scan tiles
N_J = W // P              # 4 w-groups
CE = B * HE               # 1024 transposed lines per j-group
C = B * HS                # 1024 interior (b,h) elements per j-group
NG = 4                    # PSUM evacuation groups per j (2 tiles each)

_BUILT = None
LAST_RESULTS = []   # kept for the test harness's profiling hook


def _k5_body(tc, out_d, st_d, xs_d):
    """Fused single-launch banded-EDT device pass.

    xs_d:  [16, HE, 512] f16 dram (ExternalInput, host-binarized h-slab)
    out_d: [512, 16, HS] f16 dram (ExternalOutput), squared distances,
           w-major.  Rows h in {0,1,62,63} of each slab carry garbage (no
    halo); the host re-solves them from st_d.
    st_d:  [512, 16, 8] f16 dram (ExternalOutput): the squared W-distances
           of rows h in {0..3, 60..63} (what the host boundary fix needs).
    """
    import concourse.mybir as mybir

    nc = tc.nc
    f16 = mybir.dt.float16
    Alu = mybir.AluOpType
    Act = mybir.ActivationFunctionType

    from concourse.masks import make_identity

    with tc.tile_pool(name="const", bufs=1) as cpool, \
         tc.tile_pool(name="big", bufs=1) as bpool, \
         tc.tile_pool(name="htmp", bufs=12) as hpool, \
         tc.tile_pool(name="btmp", bufs=6) as tbpool, \
         tc.tile_pool(name="psum", bufs=4, space="PSUM") as ppool, \
         tc.tile_pool(name="psumw", bufs=1, space="PSUM") as ppoolw:

        ones = cpool.tile([P, W], f16)
        nc.gpsimd.memset(ones[:], 1.0)
        ident = cpool.tile([P, P], f16)
        make_identity(nc, ident[:])
        # dummy transpose so PE observes the gpsimd-built identity before the
        # real transposes (keeps every matmul at <= 1 semaphore wait)
        psw = ppoolw.tile([P, P], f16)
        nc.tensor.transpose(psw[:], ident[:], ident[:])

        AALL = bpool.tile([P, N_T * W], f16)    # d0 = (x != 0) * CLAMP (host)
        FALL = bpool.tile([P, N_T * W], f16)    # fwd scan
        DALL = bpool.tile([P, N_T * W], f16)    # bwd scan of fwd = 1D dist

        # input DMAs on the sync queue (its trigger issue starts immediately;
        # the Pool queue is busy building the identity).  The host sends the
        # already-binarized (x != 0) * CLAMP field, so the first scan starts
        # as soon as the first (single-tile) chunk lands.
        xflat = xs_d.rearrange("b h w -> (b h) w")
        chunks = [(0, 1), (1, 2), (3, 2), (5, 3)]
        for t0, k in chunks:
            if k == 1:
                nc.sync.dma_start(AALL[:, W * t0: W * (t0 + 1)],
                                  xflat[P * t0: P * (t0 + 1)])
            else:
                nc.sync.dma_start(
                    AALL[:, W * t0: W * (t0 + k)].rearrange(
                        "p (g w) -> p g w", g=k),
                    xflat[P * t0: P * (t0 + k)].rearrange(
                        "(g pp) w -> pp g w", g=k))

        for t in range(N_T):
            fa = FALL[:, W * t: W * (t + 1)]
            nc.vector.tensor_tensor_scan(
                fa, ones[:, 0:W], AALL[:, W * t: W * (t + 1)], CLAMP,
                Alu.add, Alu.min)
            nc.vector.tensor_tensor_scan(
                DALL[:, W * t: W * (t + 1)][:, ::-1], ones[:, 0:W],
                fa[:, ::-1], CLAMP, Alu.add, Alu.min)

        # transpose + evacuate-with-Square, in NG groups of 4 scan tiles per
        # j-group; group g holds exactly the b-half g (4*128 = 8*64 lines).
        SQ = bpool.tile([P, N_J * CE], f16)     # w lines x (j, b, h64)
        GT = N_T // NG                          # 4 tiles per group
        for g in range(NG):
            for j in range(N_J):
                ps = ppool.tile([P, GT * P], f16, tag="ps")
                for tt in range(GT):
                    t = g * GT + tt
                    nc.tensor.transpose(
                        ps[:, P * tt: P * (tt + 1)],
                        DALL[:, W * t + P * j: W * t + P * (j + 1)],
                        ident[:])
                nc.scalar.activation(
                    SQ[:, CE * j + GT * P * g: CE * j + GT * P * (g + 1)],
                    ps[:], Act.Square)

        sq5 = SQ[:].rearrange("p (j b h) -> p j b h", j=N_J, b=B)
        ACH = bpool.tile([P, N_J * C], f16)
        ah4 = ACH[:].rearrange("p (j b h) -> p j b h", j=N_J, b=B)
        ACC = bpool.tile([P, N_J * C], f16)
        ac4 = ACC[:].rearrange("p (j b h) -> p j b h", j=N_J, b=B)
        # rows h in {0,1,62,63} are host-re-solved; give them a defined value
        # so the B pass math on those columns stays finite.
        nc.gpsimd.memset(ACH[:], 1024.0)

        # the squared-W-distance strips the host boundary fix needs: rows
        # h in {0..3} and {60..63}.  SWDGE (Pool queue) keeps these off the
        # 8 HWDGE lanes; they are host-bound, not device-critical.
        stv = st_d.rearrange("(j p) b e -> p j b e", p=P)
        for j in range(N_J):
            nc.gpsimd.dma_start(stv[:, j, :, 0:4], sq5[:, j, :, 0:4])
            nc.gpsimd.dma_start(stv[:, j, :, 4:8], sq5[:, j, :, HS - 4:HS])

        # b-halves align with the two PSUM evacuation groups.
        BCH = [(0, 8), (8, 16)]
        HV = HS - 2 * R           # 60 valid output rows, h in [R, HS-R)

        def h_pairs(bc, jh):
            """Pass H pair mins (DVE) + in-place +s^2 (ACT) on one
            (b-half, j-half).  All pairs are emitted before any fold so the
            DVE fold chains never starve."""
            b0, b1 = BCH[bc]
            nb = b1 - b0
            sq = sq5[:, 2 * jh:2 * (jh + 1), b0:b1, :]
            ts = []
            for s in range(1, R + 1):
                lo = sq[:, :, :, R - s:R - s + HV]
                hi = sq[:, :, :, R + s:R + s + HV]
                t_ = hpool.tile([P, 2 * nb * HV], f16, tag="hq")
                tv = t_[:].rearrange("p (j b h) -> p j b h", j=2, b=nb)
                nc.vector.tensor_tensor(tv, lo, hi, Alu.min)
                ts.append(tv)
            return ts

        def h_adds(ts):
            for s in range(1, R + 1):
                nc.scalar.activation(ts[s - 1], ts[s - 1], Act.Copy,
                                     bias=float(s * s))

        def h_folds(bc, jh, ts):
            """Pass H fold chain (DVE) on one (b-half, j-half)."""
            b0, b1 = BCH[bc]
            sq = sq5[:, 2 * jh:2 * (jh + 1), b0:b1, :]
            a = ah4[:, 2 * jh:2 * (jh + 1), b0:b1, R:R + HV]
            ctr = sq[:, :, :, R:R + HV]
            nc.vector.tensor_tensor(a, ts[0], ctr, Alu.min)
            for s in range(2, R + 1):
                nc.vector.tensor_tensor(a, ts[s - 1], a, Alu.min)

        def b_adds(j0, nj):
            """ACT-side prep for pass B on j-groups [j0, j0+nj): the b = B-1
            strip of the accumulator (its center term) and the shared +s^2
            tensors for s >= 2 (s = 1 is a DVE 4x tensor_scalar in
            b_folds)."""
            a = ah4[:, j0:j0 + nj]
            c = ac4[:, j0:j0 + nj]
            nc.scalar.activation(c[:, :, B - 1:B, :], a[:, :, B - 1:B, :],
                                 Act.Copy, bias=0.0)
            tbs = []
            for s in range(2, R + 1):
                tb = tbpool.tile([P, nj * C], f16, tag=f"tb{nj}")
                tv = tb[:].rearrange("p (j b h) -> p j b h", j=nj, b=B)
                nc.scalar.activation(tv, a, Act.Copy, bias=float(s * s))
                tbs.append(tv)
            return tbs

        def b_folds(j0, nj, tbs, h0=0, h1=HS):
            """Pass B directional folds on j-groups [j0, j0+nj), h-range
            [h0, h1) (DVE).  The s=1 add runs on DVE (4x) so the chain
            starts without ACT."""
            a = ah4[:, j0:j0 + nj, :, h0:h1]
            c = ac4[:, j0:j0 + nj, :, h0:h1]
            tb1 = tbpool.tile([P, nj * B * (h1 - h0)], f16, tag=f"tbd{nj}")
            t1 = tb1[:].rearrange("p (j b h) -> p j b h", j=nj, b=B)
            nc.vector.tensor_scalar(t1, a, 1.0, None, Alu.add)
            for s in range(1, R + 1):
                tv = t1 if s == 1 else tbs[s - 2][:, :, :, h0:h1]
                bc = B - s
                if s == 1:
                    nc.vector.tensor_tensor(c[:, :, 0:bc, :],
                                            tv[:, :, s:B, :],
                                            a[:, :, 0:bc, :], Alu.min)
                else:
                    nc.vector.tensor_tensor(c[:, :, 0:bc, :],
                                            tv[:, :, s:B, :],
                                            c[:, :, 0:bc, :], Alu.min)
                nc.vector.tensor_tensor(c[:, :, s:B, :], tv[:, :, 0:bc, :],
                                        c[:, :, s:B, :], Alu.min)

        outd = out_d.rearrange("(j p) b h -> p j (b h)", p=P)
        accs = ACC[:].rearrange("p (j c) -> p j c", j=N_J)

        # DVE order keeps the engine stall-free: every chunk's pairs first
        # (they only depend on evacuations), then the jh0 fold chains, the
        # jh0 B pass + its DMA, the jh1 fold chains, then per-j B chains
        # each followed by its own DMA so the tail drains incrementally.
        ts = {}
        for bc, jh in [(0, 0), (0, 1), (1, 0), (1, 1)]:
            ts[(bc, jh)] = h_pairs(bc, jh)
        for bc, jh in [(0, 0), (1, 0), (0, 1), (1, 1)]:
            h_adds(ts[(bc, jh)])
        h_folds(0, 0, ts[(0, 0)])
        h_folds(1, 0, ts[(1, 0)])
        tbs0 = b_adds(0, 2)
        b_folds(0, 2, tbs0)
        nc.sync.dma_start(outd[:, 0:2], accs[:, 0:2])
        h_folds(0, 1, ts[(0, 1)])
        h_folds(1, 1, ts[(1, 1)])
        tbs23 = b_adds(2, 2)
        b_folds(2, 1, [t[:, 0:1] for t in tbs23])
        nc.sync.dma_start(outd[:, 2:3], accs[:, 2:3])
        b_folds(3, 1, [t[:, 1:2] for t in tbs23])
        nc.sync.dma_start(outd[:, 3:4], accs[:, 3:4])


def _split_multi_waits(nc):
    """Walrus in this toolchain encodes at most ONE sync wait per hardware
    instruction.  Hoist extra waits onto same-engine NoOp carriers inserted
    immediately before the over-subscribed instruction (program order on the
    engine preserves the semantics exactly)."""
    import concourse.mybir as mybir

    n = 0
    for fn in nc.m.functions:
        for blk in fn.blocks:
            insts = blk.instructions
            out = []
            for inst in insts:
                si = inst.sync_info
                if si is not None and len(si.on_wait) > 1:
                    waits = list(si.on_wait)
                    for w in waits[:-1]:
                        nop = mybir.InstNoOp(
                            name=f"waitsplit-{n}", ins=[], outs=[])
                        n += 1
                        nop.engine = inst.engine
                        nop.sync_info = mybir.SyncInfo(
                            on_wait=[w], on_update=[])
                        out.append(nop)
                    inst.sync_info = mybir.SyncInfo(
                        on_wait=[waits[-1]], on_update=list(si.on_update))
                out.append(inst)
            blk.instructions = out
    return n


def _make_tc_class():
    """TileContext whose kernel-tail drain is split into one drain per proc.

    The stock tail emits a single sync-engine Drain waiting on every
    outstanding processor; this walrus build only encodes ONE sync wait per
    instruction, so the aggregated drain fails codegen.  Semantics are
    identical — the waits just land on consecutive Drain instructions.
    """
    import concourse.tile as tile
    from concourse.vector_clock import ScopedClock, VectorClock

    class SplitDrainTileContext(tile.TileContext):
        def _drain_and_barrier(self, tick_clock, wait_clock):
            gvc = tick_clock.global_clock
            for proc in range(len(gvc)):
                t = gvc[proc]
                if t <= 0:
                    continue
                d = self.nc.sync.drain()
                sv = VectorClock([0] * len(gvc))
                sv.require_at_least(proc, t)
                wait_clock.add_sem_waits(d.ins, ScopedClock({None: sv}))
            self.nc.all_engine_barrier()
            assert self.sems is not None
            popped = self.nc._tile_sem_poison_stack.pop()
            assert popped is self._sem_poison
            self.nc.clear_and_free_semaphores(
                list(self.sems.allocated().values()))
            self.nc.all_engine_barrier()

    return SplitDrainTileContext


def _build():
    """Build the fused Bass module (done once per process)."""
    import concourse.bass as bass
    import concourse.mybir as mybir

    f16 = mybir.dt.float16
    TC = _make_tc_class()

    nc5 = bass.Bass("TRN2", debug=False, num_devices=NCORES)
    xs5_d = nc5.dram_tensor("xs5", [B, HE, W], f16,
                            kind="ExternalInput").ap()
    ot5_d = nc5.dram_tensor("ot5", [W, B, HS], f16,
                            kind="ExternalOutput").ap()
    st5_d = nc5.dram_tensor("st5", [W, B, 8], f16,
                            kind="ExternalOutput").ap()
    with TC(nc5) as tc:
        _k5_body(tc, ot5_d, st5_d, xs5_d)
    _split_multi_waits(nc5)
    return (nc5,)


def _host_exact_edt(x):
    """Exact host fallback: banded numpy EDT with growing radius (f32)."""
    INF = np.float32(1e9)
    r = 8
    while True:
        d0 = np.where(x != 0, INF, np.float32(0.0))
        fwd = np.empty_like(d0)
        st = np.full(d0.shape[:2], INF, np.float32)
        for w in range(W):
            st = np.minimum(st + 1.0, d0[:, :, w]); fwd[:, :, w] = st
        st = np.full(d0.shape[:2], INF, np.float32)
        bwd = np.empty_like(d0)
        for w in range(W - 1, -1, -1):
            st = np.minimum(st + 1.0, d0[:, :, w]); bwd[:, :, w] = st
        d2 = np.minimum(fwd, bwd) ** 2
        for axis in (0, 1):
            src = d2
            acc = src.copy()
            rr = min(r, x.shape[axis] - 1)
            for s in range(1, rr + 1):
                sl_lo = [slice(None)] * 3
                sl_hi = [slice(None)] * 3
                sl_lo[axis] = slice(0, x.shape[axis] - s)
                sl_hi[axis] = slice(s, None)
                np.minimum(acc[tuple(sl_lo)], src[tuple(sl_hi)] + s * s,
                           out=acc[tuple(sl_lo)])
                np.minimum(acc[tuple(sl_hi)], src[tuple(sl_lo)] + s * s,
                           out=acc[tuple(sl_hi)])
            d2 = acc
        out = np.sqrt(d2)
        # exact when every per-axis offset fits in the band; r >= max dim
        # means the bands are complete regardless of the value of out
        if out.max() <= r or r >= max(x.shape):
            return out.astype(np.float32)
        r *= 2


_RUNNER = None


def _make_runner(nc, n_cores):
    """Build the sharded PJRT callable once (run_bass_kernel_spmd re-traces
    and re-jits on every call; caching saves ~1 s per kernel() invocation)."""
    import jax
    import numpy as _np
    from jax.sharding import Mesh, PartitionSpec
    from jax.experimental.shard_map import shard_map
    import concourse.mybir as mybir
    from concourse import bass2jax

    bass2jax.install_neuronx_cc_hook()
    partition_name = (nc.partition_id_tensor.name
                      if nc.partition_id_tensor else None)
    in_names, out_names, out_avals, zero_outs = [], [], [], []
    for alloc in nc.m.functions[0].allocations:
        if not isinstance(alloc, mybir.MemoryLocationSet):
            continue
        name = alloc.memorylocations[0].name
        if alloc.kind == "ExternalInput":
            if name != partition_name:
                in_names.append(name)
        elif alloc.kind == "ExternalOutput":
            out_avals.append(jax.core.ShapedArray(
                tuple(alloc.tensor_shape), mybir.dt.np(alloc.dtype)))
            out_names.append(name)
            zero_outs.append(_np.zeros(tuple(alloc.tensor_shape),
                                       mybir.dt.np(alloc.dtype)))
    all_in = list(in_names) + list(out_names)
    if partition_name is not None:
        all_in.append(partition_name)

    def _body(*args):
        operands = list(args)
        if partition_name is not None:
            operands.append(bass2jax.partition_id_tensor())
        return tuple(bass2jax._bass_exec_p.bind(
            *operands, out_avals=tuple(out_avals), in_names=tuple(all_in),
            out_names=tuple(out_names), lowering_input_output_aliases=(),
            sim_require_finite=True, sim_require_nnan=True, nc=nc))

    devices = jax.devices()[:n_cores]
    mesh = Mesh(_np.asarray(devices), ("core",))
    n_io = len(in_names) + len(out_names)
    fn = jax.jit(shard_map(_body, mesh=mesh,
                           in_specs=(PartitionSpec("core"),) * n_io,
                           out_specs=(PartitionSpec("core"),) * len(out_names),
                           check_rep=False), keep_unused=True)

    def run(in_maps):
        concat_in = [_np.concatenate([_np.asarray(in_maps[c][n])
                                      for c in range(n_cores)], axis=0)
                     for n in in_names]
        concat_zero = [_np.zeros((n_cores * z.shape[0], *z.shape[1:]), z.dtype)
                       for z in zero_outs]
        outs = fn(*concat_in, *concat_zero)
        return [{name: _np.asarray(outs[i]).reshape(
                    n_cores, *out_avals[i].shape)[c]
                 for i, name in enumerate(out_names)}
                for c in range(n_cores)]

    return run


def _fix_boundaries(d2, results):
    """Re-solve the 4 rows around every slab boundary (and the 2 rows at
    each volume edge) from the exported squared-W-distance strips: the
    device ran the H pass without halo, so those rows are garbage.  This is
    the same radius-R H+B min-plus the device does, just in numpy on 32 of
    512 rows."""
    # dw2[b, h, w] for the strip rows: slab k locals {0..3} u {60..63}
    dw2 = np.full((B, H, W), np.float32(np.inf))
    for k in range(NCORES):
        st = np.asarray(results[k]["st5"]).astype(np.float32)  # [W, B, 8]
        st = st.transpose(1, 2, 0)                             # [B, 8, W]
        dw2[:, k * HS:k * HS + 4, :] = st[:, 0:4, :]
        dw2[:, k * HS + HS - 4:k * HS + HS, :] = st[:, 4:8, :]
    rows = sorted({r for m in range(NCORES + 1)
                   for r in (m * HS - 2, m * HS - 1, m * HS, m * HS + 1)
                   if 0 <= r < H})
    for hg in rows:
        acc = None
        for dh in range(-R, R + 1):
            hh = hg + dh
            if not 0 <= hh < H:
                continue
            cand = dw2[:, hh, :] + np.float32(dh * dh)
            acc = cand if acc is None else np.minimum(acc, cand)
        # pass B (radius R) along the batch axis
        accb = acc.copy()
        for db in range(1, R + 1):
            np.minimum(accb[:-db], acc[db:] + db * db, out=accb[:-db])
            np.minimum(accb[db:], acc[:-db] + db * db, out=accb[db:])
        d2[:, hg, :] = accb


def _patch_far(d2, xin):
    """Re-solve every voxel with banded d^2 >= (R+1)^2 exactly via a
    radius-6 window search (any voxel the band could have gotten wrong is in
    this set: the banded value only over-estimates, and a band violation
    implies true distance >= R+1).  Returns (patched d2, ok); ok=False means
    some such voxel has no zero within distance < 6 (or there are
    implausibly many) and the caller must use the full exact fallback."""
    sus = np.argwhere(d2 >= (R + 1) ** 2 - 0.5)
    if sus.shape[0] == 0:
        return d2, True
    if sus.shape[0] > 1_000_000:
        return d2, False
    rr = 6
    zp = np.pad(xin == 0, rr, constant_values=False)
    og = np.arange(-rr, rr + 1, dtype=np.int32)
    ob, oh, ow = np.meshgrid(og, og, og, indexing="ij")
    w2 = (ob * ob + oh * oh + ow * ow).astype(np.float32).ravel()
    obf = (ob.ravel() + rr)[None, :]
    ohf = (oh.ravel() + rr)[None, :]
    owf = (ow.ravel() + rr)[None, :]
    vals = np.empty(sus.shape[0], np.float32)
    CH = 2048
    for i0 in range(0, sus.shape[0], CH):
        s = sus[i0:i0 + CH].astype(np.int32)
        win = zp[s[:, 0:1] + obf, s[:, 1:2] + ohf, s[:, 2:3] + owf]
        d2w = np.where(win, w2[None, :], np.float32(1e9)).min(axis=1)
        if (d2w > 35.5).any():
            return d2, False
        vals[i0:i0 + CH] = d2w
    d2[sus[:, 0], sus[:, 1], sus[:, 2]] = vals
    return d2, True


def kernel(x):
    global _BUILT, _RUNNER
    x = np.asarray(x)
    assert x.shape == (B, H, W)
    if x.dtype != np.float32:
        x = x.astype(np.float32)

    if _BUILT is None:
        _BUILT = _build()
    (nc5,) = _BUILT
    if _RUNNER is None:
        _RUNNER = _make_runner(nc5, NCORES)
    LAST_RESULTS.clear()

    nan_mask = np.isnan(x)
    # The host binarizes: the device receives (x != 0) * CLAMP directly
    # (NaN != 0 is True, so NaN voxels are foreground, as in the reference).
    xin = (x != 0).astype(np.float32)     # 0 at zeros, 1 at foreground/NaN
    xp = (xin * CLAMP).astype(np.float16)
    in5 = [{"xs5": np.ascontiguousarray(xp[:, k * HS:k * HS + HE, :])}
           for k in range(NCORES)]
    results = _RUNNER(in5)
    outt = np.concatenate([results[k]["ot5"] for k in range(NCORES)], axis=2)

    d2 = outt.transpose(1, 2, 0).astype(np.float32)   # (w,b,h) -> (b,h,w)
    _fix_boundaries(d2, results)
    d2, ok = _patch_far(d2, xin)
    out = np.sqrt(d2) if ok else _host_exact_edt(xin)

    if nan_mask.any():
        out = np.where(nan_mask, np.float32(np.nan), out)
    return out
